# revision 44
# baseline (speedup 1.0000x reference)
"""DynamicFilter Trainium2 kernel.

Computation (per sample b):
    h  = tanh(query @ W1.T + b1)                      [B, 256]
    cw = (h @ W2.T + b2).reshape(B, C=32, K=31)       per-sample conv weights
    x[b,t,c] = sum_k cw[b,c,k] * pad(prev_attn)[b, t+k]
    out[b,t,o] = sum_c Wfc[o,c] x[b,t,c] + bfc[o]

Key algebraic fusion: fold the fc into the conv,
    Weff[b,o,s] = sum_c Wfc[o,c] cw[b,c,s]            [B, 128, 31]
    out[b,t,o]  = sum_s Weff[b,o,s] pad(prev_attn)[b, t+s] + bfc[o]
so the T-sized work is ONE matmul per (sample, 512-wide t-chunk):
    psum[128 o, 512 t] = WeffT_b[32 s, 128 o].T @ windows[32 s, 512 t]
with the windows operand streamed from SBUF tiles holding 31 shifted
replicas of each padded row plus a row of ones; the matching 32nd
stationary row holds bfc, so PSUM accumulates conv + bias exactly in
fp32 and the psum->sbuf drain is a plain dtype-narrowing copy.

The whole T-sized data path runs in bf16 (correctness gate is
rel_err < 2e-2; bf16 rounding costs ~5e-3). PSUM accumulation is fp32.

Head latency is the battle: the framework preamble + first-DMA
latency means nothing computes before ~8.5us, so the hypernet ->
conv-weight chain must be short and DMA-free.  The v0 kernel's
per-group cw gather (sbuf->sbuf DMA) cost ~8us of HWDGE round-trip
latency on the critical path; this version never lets the conv-weight
data leave the engines:

  mm1' : hT[j, b] computed directly j-on-partitions (16 small
         matmuls, lhsT = host-prepacked W1 slices), so tanh(+b1) is a
         single ACT from PSUM with a per-partition bias -- no PE
         transpose, no identity matrix.
  mm2  : W2 columns host-reordered to (slot, channel): col = 32 s + c
         (slot 31 zero-padded), so cwB[8 b, 1024] comes out in 4 wide
         matmuls.
  DVE StreamTranspose (32x32 blocks, sbuf->sbuf, no DMA):
         cwB[32, 1024] -> cwT[32, 1024] with cwT[c, 32 s + b]
         = cw[b, c, s] -- the per-sample weight matrices land
         c-on-partitions in one instruction.
  weff : 2 matmuls, one per 4-sample quad: lhsT = cwT cols
         (i: stride 1, s: stride 32) -> psum[32 i + s, o]; rhs = WfcT.
         The stt drain adds host-folded B2eff (rows 32i+s) whose row
         31 is bfc (pairs with the replica ones-row).

Main loop: 64 conv matmuls (8 samples x 8 512-column chunks),
round-robin over 6 PSUM banks, drains alternating DVE/ACT (the only
PSUM-reading engines), out-DMA on the sync HWDGE ring.  The final
sample's tiles are finer (1024 cols) and alternate scalar/sync rings
to shorten the tail.

Sharding: data-parallel over batch. 64 samples / 8 cores = 8 per core.
Weights replicated. Output written bf16 [b, o, t]; host upcasts and
returns a transposed view [B, T, O] in f32.
"""

import sys

import numpy as np

if "/opt/trn_rl_repo" not in sys.path:
    sys.path.insert(0, "/opt/trn_rl_repo")

from contextlib import ExitStack

import ml_dtypes

import concourse.bass as bass
import concourse.mybir as mybir
import concourse.tile as tile
from concourse import bacc
from concourse.ap import AP
from concourse.bass_utils import run_bass_kernel_spmd

# Problem shapes (hardcoded per contract).
B, T = 64, 4096
D, H = 1024, 256
C, K, O = 32, 31, 128
S = 32  # slot count: 31 conv taps + the folded-bias ones row
PAD = (K - 1) // 2  # 15
NCORES = 8
BPC = B // NCORES  # 8 samples per core
TCH = 512  # t-chunk (matmul moving free dim, one PSUM bank)
NT = T // TCH  # 8 chunks per sample
OCH = 2048  # out-tile column width (4 KB bf16 runs)
PCH = 2048  # replica chunk width (4 KB bf16 runs)
# PE row/psum quadrant bases only allow {0, 32, 64}, so samples group in
# threes (3 + 3 + 2) for the weff/conv stages
GROUPS = [(0, 3), (3, 3), (6, 2)]

F32 = mybir.dt.float32
BF16 = mybir.dt.bfloat16
AF = mybir.ActivationFunctionType
BF16NP = ml_dtypes.bfloat16

_CACHED = {}


def _build_nc():
    nc = bacc.Bacc(
        "TRN2", target_bir_lowering=False, debug=False, num_devices=NCORES
    )
    # leave semaphore waits on the matmuls: a wait-free LDWEIGHTS can
    # prefetch into the PE shadow registers while the previous matmul is
    # still streaming (generate_event_semaphores still enforces the
    # 1-wait-per-instruction constraint)
    nc.move_matmul_waits_to_ldweights = lambda: None

    # host-prepacked layouts: each ring's completion semaphores
    # SERIALIZE at ~2.3us per DMA, so every ring carries as few DMAs as
    # possible, ordered by need time.  st0 = [qtp | all W1 chunks] in
    # ONE sync-ring DMA; replicas spread 2-per-ring.
    st0_h = nc.dram_tensor("st0", [128, BPC * 8 + 16 * 128], BF16,
                           kind="ExternalInput")
    w2s_h = nc.dram_tensor("w2s", [128, 2 * S * C], BF16,
                           kind="ExternalInput")
    b1_h = nc.dram_tensor("b1p", [128, 2], F32, kind="ExternalInput")
    wfct_h = nc.dram_tensor("wfct", [C, O], BF16, kind="ExternalInput")
    b2t_h = nc.dram_tensor("b2t", [96, O], F32, kind="ExternalInput")
    rep_h = nc.dram_tensor("paRep", [len(GROUPS), 96, T], BF16,
                           kind="ExternalInput")
    out_h = nc.dram_tensor("out", [BPC, O, T], BF16, kind="ExternalOutput")

    with tile.TileContext(nc) as tc:
        _emit(tc, st0_h, w2s_h, b1_h, wfct_h, b2t_h, rep_h, out_h)

    nc.compile()
    return nc


def _emit(tc, st0_h, w2s_h, b1_h, wfct_h, b2t_h, rep_h, out_h):
    nc = tc.nc
    with ExitStack() as ctx:
        singles = ctx.enter_context(tc.tile_pool(name="singles", bufs=1))
        wg_pool = ctx.enter_context(tc.tile_pool(name="wg", bufs=3))
        pa_pool = ctx.enter_context(tc.tile_pool(name="pa", bufs=6))
        out_pool = ctx.enter_context(tc.tile_pool(name="outsb", bufs=6))
        # hypernet psum: ph -> cw halves -> weff quads rotate in 2 banks
        psum_pre = ctx.enter_context(
            tc.tile_pool(name="psum_pre", bufs=2, space="PSUM")
        )
        psum_main = ctx.enter_context(
            tc.tile_pool(name="psum_main", bufs=6, space="PSUM")
        )

        # ---- staging.  Rings share the DMA engines and HBM bandwidth,
        # and each ring's completion semaphores serialize behind its
        # data, so critical weights go FIRST on each ring: sync = st0,
        # pa g0, pa g1 (+ out tiles); scalar = w2s, pa g2 (+ out
        # tiles); gpsimd = b1, wfct, b2t (all tiny).
        st0_sb = singles.tile([128, BPC * 8 + 16 * 128], BF16)
        nc.sync.dma_start(st0_sb[:], st0_h.ap())
        qt_sb = st0_sb[:, 0 : BPC * 8]

        def w1sl(dc, jc):
            off = BPC * 8 + 256 * dc + 128 * jc
            return st0_sb[:, off : off + 128]

        w2s_sb = singles.tile([128, 2 * S * C], BF16)
        nc.scalar.dma_start(w2s_sb[:], w2s_h.ap())
        b1_sb = singles.tile([128, 2], F32)
        nc.gpsimd.dma_start(b1_sb[:], b1_h.ap())
        wfct_sb = singles.tile([C, O], BF16)
        nc.gpsimd.dma_start(wfct_sb[:], wfct_h.ap())
        b2t_sb = singles.tile([96, O], F32)
        nc.gpsimd.dma_start(b2t_sb[:], b2t_h.ap())

        rep_ap = rep_h.ap()
        pa_tiles = [
            [
                pa_pool.tile([96, PCH], BF16, tag="pa", name=f"pa_g{g}c{c}")
                for c in range(T // PCH)
            ]
            for g in range(len(GROUPS))
        ]
        pa_eng = [nc.sync, nc.scalar, nc.sync]
        for g, (b0, cnt) in enumerate(GROUPS):
            for ch in range(T // PCH):
                pa_eng[g].dma_start(
                    pa_tiles[g][ch][0 : S * cnt, :],
                    rep_ap[g, 0 : S * cnt, PCH * ch : PCH * ch + PCH],
                )

        # cwB rows 8..31 are never written by mm2's drain but ARE read by
        # the stream transpose (their transposed columns are unused) --
        # memset them so the sim never sees uninitialized reads
        cwB_sb = singles.tile([S, S * C], BF16)
        cwT_sb = singles.tile([S, S * C], BF16)
        # (whole tile: engine ops need 32-aligned start partitions; the
        # mm2 drains overwrite rows 0..7 afterwards)
        nc.gpsimd.memset(cwB_sb[:], 0.0)

        # ---- hypernet mm1' (transposed orientation): hT[j, b] ----------
        # hT[128 jc + jj, b] = sum_d W1[j, d] qT[d, b]; lhsT = W1 slices.
        # Two accumulation chains interleave dc-major (so chunk-0 work
        # starts before st0b lands); separate psum tiles keep their
        # zero regions apart.
        ph = [
            psum_pre.tile([128, BPC], F32, tag="pre", name=f"ph{jc}")
            for jc in range(2)
        ]
        for n, dc in enumerate(range(8)):
            for jc in range(2):
                nc.tensor.matmul(
                    ph[jc][:],
                    lhsT=w1sl(dc, jc),
                    rhs=qt_sb[:, BPC * dc : BPC * dc + BPC],
                    start=(n == 0),
                    stop=(n == 7),
                )
        htr_sb = singles.tile([128, 2 * BPC], BF16)
        for jc in range(2):
            nc.scalar.activation(
                htr_sb[:, BPC * jc : BPC * jc + BPC],
                ph[jc][:],
                AF.Tanh,
                bias=b1_sb[:, jc : jc + 1],
            )

        # ---- hypernet mm2: cwB[b, 32 s + c] = sum_h hT[h, b] W2s[h, sc]
        # (w2s columns host-reordered (slot, channel), slot 31 zeroed;
        # b2 is host-folded into B2eff and added at the stt drain)
        HW = S * C // 2  # 512
        cwp = []
        for half in range(2):
            pc = psum_pre.tile([BPC, HW], F32, tag="pre", name=f"cw{half}")
            cwp.append(pc)
            for hc in range(2):
                nc.tensor.matmul(
                    pc[:],
                    lhsT=htr_sb[:, BPC * hc : BPC * hc + BPC],
                    rhs=w2s_sb[:, S * C * hc + HW * half :
                               S * C * hc + HW * half + HW],
                    start=(hc == 0),
                    stop=(hc == 1),
                )
        # drain half 0 on DVE, half 1 on ACT, so DVE can start the block
        # transpose of half 0 while ACT still drains half 1
        nc.vector.tensor_copy(cwB_sb[0:BPC, 0:HW], cwp[0][:])
        nc.scalar.activation(cwB_sb[0:BPC, HW : 2 * HW], cwp[1][:],
                             AF.Identity)

        # ---- 32x32 block transpose on DVE: cwT[c, 32 s + b] = cw[b,c,s]
        # (two halves: blocks are independent, so the first overlaps the
        # second mm2 chain's drain)
        nc.vector.transpose(cwT_sb[:, 0:HW], cwB_sb[:, 0:HW])
        nc.vector.transpose(cwT_sb[:, HW : 2 * HW], cwB_sb[:, HW : 2 * HW])

        # ---- weff: one matmul per sample (the stationary AP must be a
        # single free dim: cwT cols {32 s + b} at stride 32) ------------
        # psum[32 i + s, o] = sum_c cwT[c, 32 s + (b0 + i)] WfcT[c, o]
        cwT_bs = cwT_sb[:].rearrange("p (s b) -> p b s", s=S)
        wg_tiles = []
        for g, (b0, cnt) in enumerate(GROUPS):
            pw = psum_pre.tile([S * cnt, O], F32, tag="pre", name=f"pw{g}")
            for i in range(cnt):
                nc.tensor.matmul(
                    pw[S * i : S * i + S, :],
                    lhsT=cwT_bs[:, b0 + i, :],
                    rhs=wfct_sb[:],
                    start=True,
                    stop=True,
                )
            wg = wg_pool.tile([S * cnt, O], BF16, tag="weff", name=f"wg{g}")
            # wg = pw + B2eff; B2eff row 32i+31 = bfc (pairs with the
            # replica ones-row), rows 32i+s = host-folded Wfc @ b2
            nc.vector.scalar_tensor_tensor(
                wg[:], pw[:], 1.0, b2t_sb[0 : S * cnt, :],
                mybir.AluOpType.mult, mybir.AluOpType.add,
            )
            wg_tiles.append(wg)

        # ---- main loop: keep the PE stream gap-free --------------------
        idx = 0
        ntile = 0
        out_ap = out_h.ap()
        for g, (b0, cnt) in enumerate(GROUPS):
            for i in range(cnt):
                b = b0 + i
                last = b == BPC - 1
                lhsT = wg_tiles[g][S * i : S * i + S, :]
                # finer out tiles for the final sample: each chunk's DMA
                # dispatches right after its drain, alternating rings,
                # and the final drains split across both engines
                och = OCH // 2 if last else OCH
                for oc in range(T // och):
                    osb = out_pool.tile([O, och], BF16, tag="osb",
                                        name=f"osb{b}_{oc}")
                    for sub in range(och // TCH):
                        tcn = oc * (och // TCH) + sub
                        # rotate through all 8 PSUM banks: the hypernet
                        # pool's 2 banks are free once the weffs drain
                        # (same tag as the pre tiles -- pools allocate
                        # per tag)
                        if idx % 4 == 3:
                            pm = psum_pre.tile([O, TCH], F32, tag="pre")
                        else:
                            pm = psum_main.tile([O, TCH], F32, tag="pmm")
                        nc.tensor.matmul(
                            pm[:],
                            lhsT=lhsT,
                            rhs=pa_tiles[g][tcn // (PCH // TCH)][
                                S * i : S * i + S,
                                TCH * (tcn % (PCH // TCH)) :
                                TCH * (tcn % (PCH // TCH)) + TCH,
                            ],
                            start=True,
                            stop=True,
                        )
                        # psum -> sbuf bf16 narrowing copy (bias already
                        # in); only DVE and ACT can read PSUM
                        dst = osb[:, TCH * sub : TCH * sub + TCH]
                        if last and tcn >= NT - 2:
                            # tail: halve drain latency via both engines
                            nc.vector.tensor_copy(
                                dst[:, 0 : TCH // 2], pm[:, 0 : TCH // 2]
                            )
                            nc.scalar.activation(
                                dst[:, TCH // 2 : TCH],
                                pm[:, TCH // 2 : TCH], AF.Identity,
                            )
                        elif idx % 2 == 0:
                            nc.vector.tensor_copy(dst, pm[:])
                        else:
                            nc.scalar.activation(dst, pm[:], AF.Identity)
                        idx += 1
                    # alternate out tiles across the two HWDGE rings --
                    # one ring alone saturates under 8.4 MB of output
                    out_eng = nc.sync if ntile % 2 == 0 else nc.scalar
                    ntile += 1
                    out_eng.dma_start(
                        out_ap[b, :, och * oc : och * oc + och], osb[:]
                    )


def get_nc(use_f32r=True):
    # use_f32r kept for test-harness compat; the data path is bf16.
    if "nc" not in _CACHED:
        _CACHED["nc"] = _build_nc()
    return _CACHED["nc"]


def make_in_maps(query, prev_attn, W1, b1, W2, b2, Wfc, bfc):
    """Shard + lay out host inputs for the 8 cores."""
    f = np.float32
    query = np.asarray(query, f)
    prev_attn = np.asarray(prev_attn, f)
    W1 = np.asarray(W1, f)  # [H, D]
    W2 = np.asarray(W2, f)  # [C*K, H]
    Wfc = np.asarray(Wfc, f)  # [O, C]
    b1 = np.asarray(b1, f)
    b2 = np.asarray(b2, f)
    bfc = np.asarray(bfc, f)

    # w1p[p, (dc, jc, jj)] = W1[128 jc + jj, 128 dc + p]
    w1p = np.ascontiguousarray(
        W1.reshape(2, 128, 8, 128).transpose(3, 2, 0, 1).reshape(128, 16 * 128)
    ).astype(BF16NP)
    # w2s[p, (hc, s, c)] = W2[31 c + s, 128 hc + p] for s < 31, else 0
    w2r = W2.reshape(C, K, H)  # [c, k, h]
    w2s = np.zeros((128, 2 * S * C), f)
    for hc in range(2):
        blk = w2r[:, :, 128 * hc : 128 * hc + 128]  # [c, k, p]
        # dst col within half: 32*s + c
        dst = w2s[:, S * C * hc : S * C * (hc + 1)].reshape(128, S, C)
        dst[:, :K, :] = blk.transpose(2, 1, 0)  # [p, k, c]
    w2s = np.ascontiguousarray(w2s).astype(BF16NP)
    b1p = np.ascontiguousarray(b1.reshape(2, 128).T)  # [128, 2]
    wfct = np.ascontiguousarray(Wfc.T).astype(BF16NP)  # [C, O]
    # b2t[32 i + s, o]: s < 31 -> (Wfc @ b2.reshape(C, K))[o, s]; s == 31
    # -> bfc[o]
    b2eff = Wfc @ b2.reshape(C, K)  # [O, K]
    b2t = np.zeros((96, O), f)
    for i in range(3):
        b2t[S * i : S * i + K] = b2eff.T
        b2t[S * i + K] = bfc
    b2t = np.ascontiguousarray(b2t)

    in_maps = []
    for core in range(NCORES):
        sl = slice(core * BPC, (core + 1) * BPC)
        qT = query[sl].T  # [D, BPC]
        qtp = np.ascontiguousarray(
            qT.reshape(8, 128, BPC).transpose(1, 0, 2).reshape(128, 8 * BPC)
        ).astype(BF16NP)
        st0 = np.ascontiguousarray(np.concatenate([qtp, w1p], axis=1))
        # shifted replicas: paRep[g, 32 i + s, t] = pad(prev_attn)[b, s+t]
        # with row 32 i + 31 = ones (pairs with the bfc row in Weff)
        padded = np.zeros((BPC, T + 2 * PAD), f)
        padded[:, PAD : PAD + T] = prev_attn[sl]
        win = np.lib.stride_tricks.sliding_window_view(padded, T, axis=1)
        # win[b, s, t] = padded[b, s + t], s in [0, 31)
        rep = np.zeros((len(GROUPS), 96, T), BF16NP)
        for g, (b0, cnt) in enumerate(GROUPS):
            for i in range(cnt):
                rep[g, S * i : S * i + K] = win[b0 + i].astype(BF16NP)
                rep[g, S * i + K] = BF16NP(1.0)
        in_maps.append(
            {
                "st0": st0,
                "w2s": w2s,
                "b1p": b1p,
                "wfct": wfct,
                "b2t": b2t,
                "paRep": rep,
            }
        )
    return in_maps


def assemble_output(results):
    """[8 cores] x [BPC, O, T] bf16 -> [B, T, O] f32 view."""
    full = np.concatenate(
        [r["out"].astype(np.float32) for r in results], axis=0
    )  # [B, O, T]
    return full.transpose(0, 2, 1)


def kernel(query, prev_attn, W1, b1, W2, b2, Wfc, bfc):
    nc = get_nc()
    in_maps = make_in_maps(query, prev_attn, W1, b1, W2, b2, Wfc, bfc)
    res = run_bass_kernel_spmd(nc, in_maps, list(range(NCORES)))
    return assemble_output(res.results)
